# revision 39
# baseline (speedup 1.0000x reference)
"""Trainium2 Bass kernel for nn_DotProductAttention_15238543966794.

reference:
    A = state_output @ Q.T          # [B, S]
    M = A @ K                       # [B, N]
    W[i] = M @ FV[i]                # [B, D] per i  (stack over i -> [B, B, D])
    W = softmax(W, axis=-1)
    ctx[i, d] = sum_b W[i, b, d] * FV[i, b, d]
    returns (ctx [B, D], W [B, B, D])

Sharding: stack axis i (first axis of FV / W) split across 8 cores
(32 i's per core); Q, K, state_output replicated. All reductions local.

Device pipeline per core:
  prologue (fp32-exact matmuls, natural layouts so no big transposes):
    R^T[t, n] = sum_s Q[s, t] K[s, n]         (lhsT=Q, rhs=K)
    state^T  via 16 PE 128x128 transposes
    M^T[n, b] = sum_t R^T[t, n] state^T[t, b]  (lhsT=R^T, rhs=state^T)
  per i:
    W_psum = M^T.T @ FV[i]   fp32 matmuls (fp32r is ~fp16-class precision,
                             too lossy for softmax logits)
    negmax = -rowmax(W_psum)                   (DVE)
    exp    = Exp(W_psum + negmax), Z = sum     (ACT, fused accum)
    rz     = 1/Z                               (DVE)
    W_out  = exp * rz                          (DVE tensor_scalar, 2x mode)
    P      = (exp * rz) * FV[i]                (DVE scalar_tensor_tensor)
    ctx   += ones.T @ P                        (PE, fp32r: plenty for ctx)
"""

import os

import numpy as np

import concourse.bacc as bacc
import concourse.bass as bass
import concourse.mybir as mybir
import concourse.tile as tile
from concourse.bass import ds, ts
from concourse.bass_utils import run_bass_kernel_spmd

B = 256   # batch == stack size
N = 256   # feature vectors per sample
D = 1024  # feature dim
S = 1024  # state dim
N_CORES = 8
I_PER_CORE = B // N_CORES  # 32

F32 = mybir.dt.float32
F32R = mybir.dt.float32r
BF16 = mybir.dt.bfloat16
P = 128

# "fp32": exact fp32 matmuls (4 cyc/row on PE).
# "bf16x3": hi/lo bf16 split, W = Mh@Fh + Ml@Fh + Mh@Fl (6 bf16 matmuls at
#           1 cyc/row vs 8 fp32-equivalents at 4 cyc/row). Drops the Ml@Fl
#           term: ~1.6e-4 absolute logit error, ~2e-4 relative on softmax
#           weights. fp32 FV is still used for the ctx contraction.
MM_MODE = os.environ.get("KERNEL_MM_MODE", "bf16x3")

LAST_RESULT = None  # BassKernelResults of the most recent run (for test.py)
_NC_CACHE = None


def _build_nc(reps=1):
    """reps>1 repeats the main loop via a HW For_i loop — identical work and
    outputs each iteration; used only to amplify wall-clock timing above the
    ~80 ms axon dispatch noise."""
    nc = bacc.Bacc(None, target_bir_lowering=False)

    fv_d = nc.dram_tensor("fv", [I_PER_CORE, N, D], F32, kind="ExternalInput")
    st_d = nc.dram_tensor("state_t", [S, B], F32, kind="ExternalInput")
    q_d = nc.dram_tensor("q", [S, S], F32, kind="ExternalInput")
    k_d = nc.dram_tensor("k", [S, N], F32, kind="ExternalInput")
    w_d = nc.dram_tensor("w", [I_PER_CORE, N, D], F32, kind="ExternalOutput")
    ctx_d = nc.dram_tensor("ctx", [I_PER_CORE, D], F32, kind="ExternalOutput")

    SO = S // P   # 8 chunks of the state/contraction dims
    NO = N // P   # 2
    DH = D // 512  # 2 psum-bank halves of D

    wps_bufs = int(os.environ.get("KERNEL_WPS_BUFS", "3"))
    ctx_bufs = int(os.environ.get("KERNEL_CTX_BUFS", "1"))
    with tile.TileContext(nc) as tc:
        with (
            tc.tile_pool(name="const", bufs=1) as const_pool,
            tc.tile_pool(name="pro", bufs=1) as pro,
            tc.tile_pool(name="fvp", bufs=4) as fvp,
            tc.tile_pool(name="sm", bufs=int(os.environ.get("KERNEL_SM_BUFS", "3"))) as smp,
            tc.tile_pool(name="pp", bufs=4) as ppp,
        ):
            ones_f = const_pool.tile([P, 1], F32)
            nc.vector.memset(ones_f[:], 1.0)
            ones = const_pool.tile([P, 1], F32R)
            nc.vector.tensor_copy(ones[:], ones_f[:])

            # ---- prologue: M^T = f(Q, K, state) ----
            q_sb = pro.tile([P, SO, S], F32)      # Q[s, t] -> [si, so, t]
            k_sb = pro.tile([P, SO, N], F32)      # K[s, n] -> [si, so, n]
            stT_sb = pro.tile([P, SO, B], F32)    # state^T[t, b] -> [ti, to, b]
            # chunked + interleaved so the so=0 R^T matmuls can start after
            # only k0+q0 land (~640KB) instead of the full 6MB
            k_re = k_d.rearrange("(so si) n -> si so n", si=P)
            q_re = q_d.rearrange("(so si) t -> si so t", si=P)
            for so in range(SO):
                nc.sync.dma_start(k_sb[:, so], k_re[:, so])
                nc.sync.dma_start(q_sb[:, so], q_re[:, so])
            nc.sync.dma_start(stT_sb[:], st_d.rearrange("(to ti) b -> ti to b", ti=P))

            rt_sb = pro.tile([P, SO, N], F32)     # [ti, to, n]
            mt_sb = pro.tile([P, NO, B], F32)     # [ni, no, b]
            with tc.tile_pool(name="pro_ps", bufs=1, space="PSUM") as pro_ps:
                # R^T[t, n] = sum_s Q[s, t] * K[s, n]; so-major so the matmuls
                # start as soon as the first q chunk lands (8 interleaved
                # accumulation groups, one per to-slice of a 4-bank tile).
                # NOTE: start=True clears has_written for the WHOLE 2KB bank,
                # so with two 1KB to-regions per bank only the bank's first
                # matmul may carry start=True; the second region's first write
                # still overwrites (its has_written bits were cleared by the
                # bank clear), later ones accumulate.
                r_ps = pro_ps.tile([P, SO, N], F32, tag="r_ps")
                for so in range(SO):
                    for to in range(SO):
                        nc.tensor.matmul(
                            r_ps[:, to], q_sb[:, so, ts(to, P)], k_sb[:, so],
                            start=(so == 0 and to % 2 == 0),
                            stop=(so == SO - 1),
                            skip_group_check=True,
                        )
                for to in range(SO):
                    nc.scalar.copy(rt_sb[:, to], r_ps[:, to])

                # M^T[n, b] = sum_t R^T[t, n] * state^T[t, b]
                m_ps = pro_ps.tile([P, NO, B], F32, tag="m_ps")
                for no in range(NO):
                    for to in range(SO):
                        nc.tensor.matmul(
                            m_ps[:, no], rt_sb[:, to, ts(no, P)], stT_sb[:, to],
                            start=(to == 0), stop=(to == SO - 1),
                        )
                    nc.scalar.copy(mt_sb[:, no], m_ps[:, no])

            if MM_MODE == "bf16x3":
                mh_sb = pro.tile([P, NO, B], BF16)
                ml_sb = pro.tile([P, NO, B], BF16)
                nc.vector.tensor_copy(mh_sb[:], mt_sb[:])
                nc.vector.tensor_tensor(ml_sb[:], mt_sb[:], mh_sb[:],
                                        mybir.AluOpType.subtract)
                m_tiles = (mh_sb, ml_sb)
            else:
                m_tiles = (mt_sb,)

            # ---- main loop over the 32 local i's ----
            import contextlib
            with (
                tc.tile_pool(name="wps_p", bufs=wps_bufs, space="PSUM") as wps_p,
                tc.tile_pool(name="ctx_ps_p", bufs=ctx_bufs, space="PSUM") as ctx_ps_p,
            ):
                loop_cm = (tc.For_i(0, reps, 1) if reps > 1
                           else contextlib.nullcontext())
                with loop_cm:
                    _main_body(nc, tc, fvp, smp, ppp, wps_p, ctx_ps_p,
                               fv_d, w_d, ctx_d, m_tiles, ones)

    nc.finalize()
    return nc


def _main_body(nc, tc, fvp, smp, ppp, wps_p, ctx_ps_p, fv_d, w_d, ctx_d, m_tiles, ones):
    NO = N // P
    DH = D // 512
    fvh_eng = os.environ.get("KERNEL_FVH", "dve")

    def dma_stage(j):
        """fv DMA for iteration j — issued two iterations ahead so the split
        engines never wait on it."""
        fv_sb = fvp.tile([P, NO, D], F32, tag="fv")
        nc.sync.dma_start(
            fv_sb[:], fv_d[j].rearrange("(no ni) d -> ni no d", ni=P)
        )
        return fv_sb

    def split_stage(j, fv_sb):
        """hi/lo bf16 split for iteration j — emitted at the END of iteration
        j-1 so it sits after that iteration's critical DVE ops but before
        iteration j's."""
        if MM_MODE != "bf16x3":
            (mt_sb,) = m_tiles
            return [(mt_sb, fv_sb)]
        fvh = fvp.tile([P, NO, D], BF16, tag="fvh", bufs=4)
        fvl = fvp.tile([P, NO, D], BF16, tag="fvl", bufs=4)
        eng = fvh_eng
        if eng == "mix":
            eng = "dve" if j % 2 == 0 else "gps"
        elif eng == "mixdma":
            eng = "dma" if j % 2 == 0 else "gps"
        if eng == "act":
            nc.scalar.copy(fvh[:], fv_sb[:])
        elif eng == "dve":
            nc.vector.tensor_copy(fvh[:], fv_sb[:])
        elif eng == "dma":
            nc.gpsimd.dma_start(
                fvh[:], fv_d[j].rearrange("(no ni) d -> ni no d", ni=P))
        else:
            nc.gpsimd.tensor_copy(fvh[:], fv_sb[:])
        fvl_eng = os.environ.get("KERNEL_FVL", "dve")
        if fvl_eng == "mix":
            fvl_eng = "dve" if j % 2 == 1 else "gps"
        if fvl_eng == "dve":
            nc.vector.tensor_tensor(fvl[:], fv_sb[:], fvh[:],
                                    mybir.AluOpType.subtract)
        else:
            nc.gpsimd.tensor_tensor(fvl[:], fv_sb[:], fvh[:],
                                    mybir.AluOpType.subtract)
        mh_sb, ml_sb = m_tiles
        return [(mh_sb, fvh), (ml_sb, fvh), (mh_sb, fvl)]

    def flush_ctx(pend):
        """ctx reduction for iteration i, deferred to i+1 so the PE matmuls
        read long-finished p tiles instead of stalling on the softmax chain."""
        j, p_tiles = pend
        ctx_ps = ctx_ps_p.tile([1, DH, 512], F32, tag="ctx_ps")
        for bc in range(NO):
            for h in range(DH):
                nc.tensor.matmul(
                    ctx_ps[0:1, h],
                    ones[:],
                    p_tiles[bc][:, ds(h * 512, 512)],
                    start=(bc == 0), stop=(bc == NO - 1),
                )
        ctx_sb = smp.tile([1, D], F32, tag="ctx_sb")
        for h in range(DH):
            nc.scalar.copy(ctx_sb[0:1, ds(h * 512, 512)], ctx_ps[0:1, h])
        nc.sync.dma_start(ctx_d[ds(j, 1)], ctx_sb[:])

    fv_tiles = {0: dma_stage(0)}
    if I_PER_CORE > 1:
        fv_tiles[1] = dma_stage(1)
    splits = {0: split_stage(0, fv_tiles[0])}
    if I_PER_CORE > 1:
        splits[1] = split_stage(1, fv_tiles[1])
    pending = None
    for i in range(I_PER_CORE):
        fv_sb = fv_tiles.pop(i)
        passes = splits.pop(i)
        if i + 2 < I_PER_CORE:
            fv_tiles[i + 2] = dma_stage(i + 2)

        p_tiles = []
        w_list = [wps_p.tile([P, D], F32, tag="wps", name=f"wps{bc}")
                  for bc in range(NO)]
        n_mm = len(passes) * NO
        # all hi-pass matmuls for both b-chunks first, then the deferred ctx
        # matmuls of i-1, then the lo-pass matmuls: maximizes the window for
        # the fvl producer before PE needs it.
        for pi, (m_t, f_t) in enumerate(passes):
            if pi == len(passes) - 1 and pending is not None and len(passes) > 1:
                flush_ctx(pending)
                pending = None
            for bc in range(NO):
                for h in range(DH):
                    for nck in range(NO):
                        mm = pi * NO + nck
                        nc.tensor.matmul(
                            w_list[bc][:, ds(h * 512, 512)],
                            m_t[:, nck, ts(bc, P)],
                            f_t[:, nck, ds(h * 512, 512)],
                            start=(mm == 0), stop=(mm == n_mm - 1),
                        )

        for bc in range(NO):
            w_ps = w_list[bc]
            negmax = smp.tile([P, 1], F32, tag="negmax")
            nc.vector.tensor_reduce(
                negmax[:], w_ps[:], axis=mybir.AxisListType.X,
                op=mybir.AluOpType.max, negate=True,
            )
            z = smp.tile([P, 1], F32, tag="z")
            exp_sb = smp.tile([P, D], F32, tag="exp")
            nc.scalar.activation(
                exp_sb[:], w_ps[:], mybir.ActivationFunctionType.Exp,
                bias=negmax[:], scale=1.0, accum_out=z[:],
            )
            rz = smp.tile([P, 1], F32, tag="rz")
            nc.vector.reciprocal(rz[:], z[:])

            # P = (exp * rz) * fv  -> ctx contribution
            p_sb = ppp.tile([P, D], F32R, tag="p")
            nc.vector.scalar_tensor_tensor(
                p_sb[:], exp_sb[:], rz[:], fv_sb[:, bc],
                op0=mybir.AluOpType.mult, op1=mybir.AluOpType.mult,
            )
            p_tiles.append(p_sb)
            # W output tile (normalized softmax weights). In bf16x3 mode DVE
            # is hotter (reduce+STT), so normalize on ACT.
            w_out = smp.tile([P, D], F32, tag="wout")
            if MM_MODE == "bf16x3":
                nc.scalar.mul(w_out[:], exp_sb[:], rz[:])
            else:
                nc.vector.tensor_scalar_mul(w_out[:], exp_sb[:], rz[:])
            nc.sync.dma_start(w_d[i, ds(bc * P, P)], w_out[:])

        if pending is not None:
            flush_ctx(pending)
        pending = (i, p_tiles)
        if i + 2 < I_PER_CORE:
            splits[i + 2] = split_stage(i + 2, fv_tiles[i + 2])

    flush_ctx(pending)


def make_in_maps(inputs):
    fv = np.ascontiguousarray(inputs["feature_vectors"], dtype=np.float32)
    state_t = np.ascontiguousarray(
        np.asarray(inputs["state_output"], dtype=np.float32).T)
    q = np.ascontiguousarray(inputs["Q"], dtype=np.float32)
    k = np.ascontiguousarray(inputs["K"], dtype=np.float32)
    in_maps = []
    for c in range(N_CORES):
        sl = slice(c * I_PER_CORE, (c + 1) * I_PER_CORE)
        in_maps.append({
            "fv": fv[sl],
            "state_t": state_t,
            "q": q,
            "k": k,
        })
    return in_maps


def kernel(feature_vectors, state_output, Q, K):
    global LAST_RESULT, _NC_CACHE
    if _NC_CACHE is None:
        _NC_CACHE = _build_nc()
    nc = _NC_CACHE

    in_maps = make_in_maps({
        "feature_vectors": feature_vectors,
        "state_output": state_output,
        "Q": Q,
        "K": K,
    })

    res = run_bass_kernel_spmd(
        nc, in_maps, core_ids=list(range(N_CORES)),
        trace=bool(int(os.environ.get("KERNEL_TRACE", "0"))),
    )
    LAST_RESULT = res

    W = np.concatenate([r["w"] for r in res.results], axis=0)
    ctx = np.concatenate([r["ctx"] for r in res.results], axis=0)
    return ctx, W


# revision 43
# speedup vs baseline: 1.1662x; 1.1662x over previous
"""Trainium2 Bass kernel for nn_DotProductAttention_15238543966794.

reference:
    A = state_output @ Q.T          # [B, S]
    M = A @ K                       # [B, N]
    W[i] = M @ FV[i]                # [B, D] per i  (stack over i -> [B, B, D])
    W = softmax(W, axis=-1)
    ctx[i, d] = sum_b W[i, b, d] * FV[i, b, d]
    returns (ctx [B, D], W [B, B, D])

Sharding: stack axis i (first axis of FV / W) split across 8 cores
(32 i's per core); Q, K, state_output replicated (state is passed
pre-transposed by the host so the prologue needs no on-device transposes).
All reductions are local to a shard.

Device pipeline per core:
  prologue (fp32-exact matmuls, natural layouts):
    R^T[t, n] = sum_s Q[s, t] K[s, n]          (lhsT=Q, rhs=K, fp32)
    M^T[n, b] = sum_t R^T[t, n] state^T[t, b]  (lhsT=R^T, rhs=state^T, fp32)
    M^T split into fp16 hi/lo pair (Mh, Ml)
  per i (software-pipelined: fv DMA issued 2 iterations ahead, the fp16
  hi/lo split of FV 1 ahead, the ctx reduction deferred 1 back so the PE
  never stalls on the softmax chain):
    FVh = fp16(FV[i]); FVl = fp16(FV[i] - FVh)       (DVE)
    W_psum = Mh@FVh + Ml@FVh + Mh@FVl   6 fp16 matmuls @1cyc/row vs fp32's
                             4cyc/row; dropped Ml@FVl term -> ~4e-5 W error
                             (fp32r direct would be ~1e-2-class: too lossy)
    negmax = -rowmax(W_psum)                   (DVE, PSUM read)
    exp, Z = Exp(W_psum + negmax), fused sum   (ACT, PSUM->SBUF)
    rz     = 1/Z                               (DVE)
    W_out  = exp * rz                          (ACT copy-with-scale)
    P      = (exp * rz) * FV[i]                (DVE scalar_tensor_tensor)
    ctx[i] = ones.T @ [P(b0); P(b1)]           (PE, fp32r: fine for ctx)

Measured (8 cores, trn2): main loop ~270-280 us/core + ~40 us prologue;
max rel err 6.5e-5. Engine busy (cost model): DVE 258, PE ~225, DMA 205,
ACT 189 us. GPSIMD is avoided entirely — measured far slower than modeled.
"""

import os

import numpy as np

import concourse.bacc as bacc
import concourse.bass as bass
import concourse.mybir as mybir
import concourse.tile as tile
from concourse.bass import ds, ts
from concourse.bass_utils import run_bass_kernel_spmd

B = 256   # batch == stack size
N = 256   # feature vectors per sample
D = 1024  # feature dim
S = 1024  # state dim
N_CORES = 8
I_PER_CORE = B // N_CORES  # 32

F32 = mybir.dt.float32
F32R = mybir.dt.float32r
BF16 = mybir.dt.bfloat16
FP16 = mybir.dt.float16
# split dtype: fp16 pairs carry 11-bit mantissas -> ~2e-5 logit error vs
# ~2.8e-4 for bf16 pairs, at identical PE cost (both 1 cyc/row)
SPLIT_DT = FP16 if os.environ.get("KERNEL_SPLIT_DT", "fp16") == "fp16" else BF16
P = 128

# "fp32": exact fp32 matmuls (4 cyc/row on PE).
# "bf16x3": hi/lo SPLIT_DT split, W = Mh@Fh + Ml@Fh + Mh@Fl (6 matmuls at
#           1 cyc/row vs 8 fp32-equivalents at 4 cyc/row); fp32 FV is still
#           used for the ctx contraction.
MM_MODE = os.environ.get("KERNEL_MM_MODE", "bf16x3")

LAST_RESULT = None  # BassKernelResults of the most recent run (for test.py)
_NC_CACHE = None


def _build_nc(reps=1):
    """reps>1 repeats the main loop via a HW For_i loop — identical work and
    outputs each iteration; used only to amplify wall-clock timing above the
    ~80 ms axon dispatch noise."""
    nc = bacc.Bacc(None, target_bir_lowering=False)

    fv_d = nc.dram_tensor("fv", [I_PER_CORE, N, D], F32, kind="ExternalInput")
    st_d = nc.dram_tensor("state_t", [S, B], F32, kind="ExternalInput")
    q_d = nc.dram_tensor("q", [S, S], F32, kind="ExternalInput")
    k_d = nc.dram_tensor("k", [S, N], F32, kind="ExternalInput")
    w_d = nc.dram_tensor("w", [I_PER_CORE, N, D], F32, kind="ExternalOutput")
    ctx_d = nc.dram_tensor("ctx", [I_PER_CORE, D], F32, kind="ExternalOutput")

    SO = S // P   # 8 chunks of the state/contraction dims
    NO = N // P   # 2
    DH = D // 512  # 2 psum-bank halves of D

    wps_bufs = int(os.environ.get("KERNEL_WPS_BUFS", "3"))
    ctx_bufs = int(os.environ.get("KERNEL_CTX_BUFS", "1"))
    with tile.TileContext(nc) as tc:
        with (
            tc.tile_pool(name="const", bufs=1) as const_pool,
            tc.tile_pool(name="pro", bufs=1) as pro,
            tc.tile_pool(name="fvp", bufs=4) as fvp,
            tc.tile_pool(name="sm", bufs=int(os.environ.get("KERNEL_SM_BUFS", "3"))) as smp,
            tc.tile_pool(name="pp", bufs=4) as ppp,
        ):
            ones_f = const_pool.tile([P, 1], F32)
            nc.vector.memset(ones_f[:], 1.0)
            ones = const_pool.tile([P, 1], F32R)
            nc.vector.tensor_copy(ones[:], ones_f[:])

            # ---- prologue: M^T = f(Q, K, state) ----
            q_sb = pro.tile([P, SO, S], F32)      # Q[s, t] -> [si, so, t]
            k_sb = pro.tile([P, SO, N], F32)      # K[s, n] -> [si, so, n]
            stT_sb = pro.tile([P, SO, B], F32)    # state^T[t, b] -> [ti, to, b]
            # chunked + interleaved so the so=0 R^T matmuls can start after
            # only k0+q0 land (~640KB) instead of the full 6MB
            k_re = k_d.rearrange("(so si) n -> si so n", si=P)
            q_re = q_d.rearrange("(so si) t -> si so t", si=P)
            for so in range(SO):
                nc.sync.dma_start(k_sb[:, so], k_re[:, so])
                nc.sync.dma_start(q_sb[:, so], q_re[:, so])
            nc.sync.dma_start(stT_sb[:], st_d.rearrange("(to ti) b -> ti to b", ti=P))

            rt_sb = pro.tile([P, SO, N], F32)     # [ti, to, n]
            mt_sb = pro.tile([P, NO, B], F32)     # [ni, no, b]
            with tc.tile_pool(name="pro_ps", bufs=1, space="PSUM") as pro_ps:
                # R^T[t, n] = sum_s Q[s, t] * K[s, n]; so-major so the matmuls
                # start as soon as the first q chunk lands (8 interleaved
                # accumulation groups, one per to-slice of a 4-bank tile).
                # NOTE: start=True clears has_written for the WHOLE 2KB bank,
                # so with two 1KB to-regions per bank only the bank's first
                # matmul may carry start=True; the second region's first write
                # still overwrites (its has_written bits were cleared by the
                # bank clear), later ones accumulate.
                r_ps = pro_ps.tile([P, SO, N], F32, tag="r_ps")
                for so in range(SO):
                    for to in range(SO):
                        nc.tensor.matmul(
                            r_ps[:, to], q_sb[:, so, ts(to, P)], k_sb[:, so],
                            start=(so == 0 and to % 2 == 0),
                            stop=(so == SO - 1),
                            skip_group_check=True,
                        )
                for to in range(SO):
                    nc.scalar.copy(rt_sb[:, to], r_ps[:, to])

                # M^T[n, b] = sum_t R^T[t, n] * state^T[t, b]
                m_ps = pro_ps.tile([P, NO, B], F32, tag="m_ps")
                for no in range(NO):
                    for to in range(SO):
                        nc.tensor.matmul(
                            m_ps[:, no], rt_sb[:, to, ts(no, P)], stT_sb[:, to],
                            start=(to == 0), stop=(to == SO - 1),
                        )
                    nc.scalar.copy(mt_sb[:, no], m_ps[:, no])

            if MM_MODE == "bf16x3":
                mh_sb = pro.tile([P, NO, B], SPLIT_DT)
                ml_sb = pro.tile([P, NO, B], SPLIT_DT)
                nc.vector.tensor_copy(mh_sb[:], mt_sb[:])
                nc.vector.tensor_tensor(ml_sb[:], mt_sb[:], mh_sb[:],
                                        mybir.AluOpType.subtract)
                m_tiles = (mh_sb, ml_sb)
            else:
                m_tiles = (mt_sb,)

            # ---- main loop over the 32 local i's ----
            import contextlib
            with (
                tc.tile_pool(name="wps_p", bufs=wps_bufs, space="PSUM") as wps_p,
                tc.tile_pool(name="ctx_ps_p", bufs=ctx_bufs, space="PSUM") as ctx_ps_p,
            ):
                loop_cm = (tc.For_i(0, reps, 1) if reps > 1
                           else contextlib.nullcontext())
                with loop_cm:
                    _main_body(nc, tc, fvp, smp, ppp, wps_p, ctx_ps_p,
                               fv_d, w_d, ctx_d, m_tiles, ones)

    nc.finalize()
    return nc


def _main_body(nc, tc, fvp, smp, ppp, wps_p, ctx_ps_p, fv_d, w_d, ctx_d, m_tiles, ones):
    NO = N // P
    DH = D // 512
    fvh_eng = os.environ.get("KERNEL_FVH", "dve")

    def dma_stage(j):
        """fv DMA for iteration j — issued two iterations ahead so the split
        engines never wait on it."""
        fv_sb = fvp.tile([P, NO, D], F32, tag="fv")
        nc.sync.dma_start(
            fv_sb[:], fv_d[j].rearrange("(no ni) d -> ni no d", ni=P)
        )
        return fv_sb

    def split_stage(j, fv_sb):
        """hi/lo bf16 split for iteration j — emitted at the END of iteration
        j-1 so it sits after that iteration's critical DVE ops but before
        iteration j's."""
        if MM_MODE != "bf16x3":
            (mt_sb,) = m_tiles
            return [(mt_sb, fv_sb)]
        fvh = fvp.tile([P, NO, D], SPLIT_DT, tag="fvh", bufs=4)
        fvl = fvp.tile([P, NO, D], SPLIT_DT, tag="fvl", bufs=4)
        eng = fvh_eng
        if eng == "mix":
            eng = "dve" if j % 2 == 0 else "act"
        elif eng == "mixdma":
            eng = "dma" if j % 2 == 0 else "gps"
        if eng == "act":
            nc.scalar.copy(fvh[:], fv_sb[:])
        elif eng == "dve":
            nc.vector.tensor_copy(fvh[:], fv_sb[:])
        elif eng == "dma":
            nc.gpsimd.dma_start(
                fvh[:], fv_d[j].rearrange("(no ni) d -> ni no d", ni=P))
        else:
            nc.gpsimd.tensor_copy(fvh[:], fv_sb[:])
        fvl_eng = os.environ.get("KERNEL_FVL", "dve")
        if fvl_eng == "mix":
            fvl_eng = "dve" if j % 2 == 1 else "gps"
        if fvl_eng == "dve":
            nc.vector.tensor_tensor(fvl[:], fv_sb[:], fvh[:],
                                    mybir.AluOpType.subtract)
        else:
            nc.gpsimd.tensor_tensor(fvl[:], fv_sb[:], fvh[:],
                                    mybir.AluOpType.subtract)
        mh_sb, ml_sb = m_tiles
        return [(mh_sb, fvh), (ml_sb, fvh), (mh_sb, fvl)]

    def flush_ctx(pend):
        """ctx reduction for iteration i, deferred to i+1 so the PE matmuls
        read long-finished p tiles instead of stalling on the softmax chain."""
        j, p_tiles = pend
        ctx_ps = ctx_ps_p.tile([1, DH, 512], F32, tag="ctx_ps")
        for bc in range(NO):
            for h in range(DH):
                nc.tensor.matmul(
                    ctx_ps[0:1, h],
                    ones[:],
                    p_tiles[bc][:, ds(h * 512, 512)],
                    start=(bc == 0), stop=(bc == NO - 1),
                )
        ctx_sb = smp.tile([1, D], F32, tag="ctx_sb")
        for h in range(DH):
            nc.scalar.copy(ctx_sb[0:1, ds(h * 512, 512)], ctx_ps[0:1, h])
        nc.sync.dma_start(ctx_d[ds(j, 1)], ctx_sb[:])

    fv_tiles = {0: dma_stage(0)}
    if I_PER_CORE > 1:
        fv_tiles[1] = dma_stage(1)
    splits = {0: split_stage(0, fv_tiles[0])}
    if I_PER_CORE > 1:
        splits[1] = split_stage(1, fv_tiles[1])
    pending = None
    for i in range(I_PER_CORE):
        fv_sb = fv_tiles.pop(i)
        passes = splits.pop(i)
        if i + 2 < I_PER_CORE:
            fv_tiles[i + 2] = dma_stage(i + 2)

        p_tiles = []
        w_list = [wps_p.tile([P, D], F32, tag="wps", name=f"wps{bc}")
                  for bc in range(NO)]
        n_mm = len(passes) * NO
        # all hi-pass matmuls for both b-chunks first, then the deferred ctx
        # matmuls of i-1, then the lo-pass matmuls: maximizes the window for
        # the fvl producer before PE needs it.
        for pi, (m_t, f_t) in enumerate(passes):
            if pi == len(passes) - 1 and pending is not None and len(passes) > 1:
                flush_ctx(pending)
                pending = None
            for bc in range(NO):
                for h in range(DH):
                    for nck in range(NO):
                        mm = pi * NO + nck
                        nc.tensor.matmul(
                            w_list[bc][:, ds(h * 512, 512)],
                            m_t[:, nck, ts(bc, P)],
                            f_t[:, nck, ds(h * 512, 512)],
                            start=(mm == 0), stop=(mm == n_mm - 1),
                        )

        for bc in range(NO):
            w_ps = w_list[bc]
            negmax = smp.tile([P, 1], F32, tag="negmax")
            nc.vector.tensor_reduce(
                negmax[:], w_ps[:], axis=mybir.AxisListType.X,
                op=mybir.AluOpType.max, negate=True,
            )
            z = smp.tile([P, 1], F32, tag="z")
            exp_sb = smp.tile([P, D], F32, tag="exp")
            nc.scalar.activation(
                exp_sb[:], w_ps[:], mybir.ActivationFunctionType.Exp,
                bias=negmax[:], scale=1.0, accum_out=z[:],
            )
            rz = smp.tile([P, 1], F32, tag="rz")
            nc.vector.reciprocal(rz[:], z[:])

            # P = (exp * rz) * fv  -> ctx contribution
            p_sb = ppp.tile([P, D], F32R, tag="p")
            nc.vector.scalar_tensor_tensor(
                p_sb[:], exp_sb[:], rz[:], fv_sb[:, bc],
                op0=mybir.AluOpType.mult, op1=mybir.AluOpType.mult,
            )
            p_tiles.append(p_sb)
            # W output tile (normalized softmax weights). In bf16x3 mode DVE
            # is hotter (reduce+STT), so normalize on ACT.
            w_out = smp.tile([P, D], F32, tag="wout")
            if MM_MODE == "bf16x3":
                nc.scalar.mul(w_out[:], exp_sb[:], rz[:])
            else:
                nc.vector.tensor_scalar_mul(w_out[:], exp_sb[:], rz[:])
            nc.sync.dma_start(w_d[i, ds(bc * P, P)], w_out[:])

        if pending is not None:
            flush_ctx(pending)
        pending = (i, p_tiles)
        if i + 2 < I_PER_CORE:
            splits[i + 2] = split_stage(i + 2, fv_tiles[i + 2])

    flush_ctx(pending)


def make_in_maps(inputs):
    fv = np.ascontiguousarray(inputs["feature_vectors"], dtype=np.float32)
    state_t = np.ascontiguousarray(
        np.asarray(inputs["state_output"], dtype=np.float32).T)
    q = np.ascontiguousarray(inputs["Q"], dtype=np.float32)
    k = np.ascontiguousarray(inputs["K"], dtype=np.float32)
    in_maps = []
    for c in range(N_CORES):
        sl = slice(c * I_PER_CORE, (c + 1) * I_PER_CORE)
        in_maps.append({
            "fv": fv[sl],
            "state_t": state_t,
            "q": q,
            "k": k,
        })
    return in_maps


def kernel(feature_vectors, state_output, Q, K):
    global LAST_RESULT, _NC_CACHE
    if _NC_CACHE is None:
        _NC_CACHE = _build_nc()
    nc = _NC_CACHE

    in_maps = make_in_maps({
        "feature_vectors": feature_vectors,
        "state_output": state_output,
        "Q": Q,
        "K": K,
    })

    res = run_bass_kernel_spmd(
        nc, in_maps, core_ids=list(range(N_CORES)),
        trace=bool(int(os.environ.get("KERNEL_TRACE", "0"))),
    )
    LAST_RESULT = res

    W = np.concatenate([r["w"] for r in res.results], axis=0)
    ctx = np.concatenate([r["ctx"] for r in res.results], axis=0)
    return ctx, W
